# revision 3
# baseline (speedup 1.0000x reference)
"""BatchBlobLoss Trainium2 kernel (8-core SPMD).

Reference computation:
  p = softmax(predictions, axis=1)[:, 1:]          # foreground class probs
  per (b, c): segment-sum of p keyed by instance id t = targets[b, c]
  soft-dice per (b, c, instance), masked mean -> scalar.

Device strategy (per core; cores k = 0..7 get batch b = k//4 and
D-slices 16*(k%4) .. +16):
  The 33-bin segment sum is computed with one fused reduce-op per bin:
    x = t + p  (p in (0,1) strictly, so floor(x) = t)
    ACT (scalar engine):  G_m  = sum relu(x - m)       = B_m + sum_{i>m} N_{>=i}
    ACT (Sign):           S_m  = sum sign(x - m)       = 2*N_{>=m} - n
    DVE (is_ge + accum):  N_{>=m} = sum [x >= m]
  where B_m = sum_{t >= m} p. Host (float64) recovers
    P_m = B_m - B_{m+1}  (per-instance prob sums) and C_m = N_{>=m} - N_{>=m+1}
  and evaluates the tiny dice formula. Per-partition accumulator strips
  [128, n_cols] are DMA'd out and reduced on host.
"""
import numpy as np
from contextlib import ExitStack

import concourse.bass as bass
import concourse.tile as tile
from concourse import bacc, mybir
from concourse import bass_utils
from concourse.bass_interp import get_hw_module

# problem shape (hardcoded per contest rules)
B, C, D, H, W = 2, 3, 64, 256, 256
M = 32
EPS = 1e-5
N_CORES = 8
CORES_PER_BATCH = 4
D_SH = D // CORES_PER_BATCH      # 16 depth slices per core
P = 128
NVOX = D_SH * H * W              # 1,048,576 voxels per core per channel
F = NVOX // P                    # 8192
CHUNK = 4096
NCHUNK = F // CHUNK              # 2
NBINS = 33                       # ids 0..32

# engine split for the 65 binning passes per (chunk, channel)
VAL_ACT = list(range(NBINS))     # value bins via ACT Relu -> G_m
CNT_DVE = list(range(1, 28))     # count bins via DVE is_ge -> N_{>=m}
CNT_ACT = list(range(28, NBINS)) # count bins via ACT Sign -> 2N - n

COLS_PER_SET = 65                # 33 value + 32 count columns
N_COLS = NCHUNK * 2 * COLS_PER_SET

F32 = mybir.dt.float32
BF16 = mybir.dt.bfloat16
I32 = mybir.dt.int32


def _val_col(chunk, ch, m):
    return (chunk * 2 + ch) * COLS_PER_SET + m


def _cnt_col(chunk, ch, m):
    return (chunk * 2 + ch) * COLS_PER_SET + NBINS + (m - 1)


def build_nc():
    AluOp = mybir.AluOpType
    ACT = mybir.ActivationFunctionType

    nc = bacc.Bacc("TRN2", target_bir_lowering=False, debug=False,
                   num_devices=N_CORES)
    pred = nc.dram_tensor("pred", [C, P, F], F32, kind="ExternalInput").ap()
    targ = nc.dram_tensor("targ", [2, P, F], I32, kind="ExternalInput").ap()
    out = nc.dram_tensor("out", [P, N_COLS], F32, kind="ExternalOutput").ap()

    with tile.TileContext(nc) as tc:
        with ExitStack() as ctx:
            pool = ctx.enter_context(tc.tile_pool(name="main", bufs=1))

            # bias strip: column m holds -m (f32), for ACT bias
            bias_i = pool.tile([P, NBINS], I32, tag="bias_i")
            nc.gpsimd.iota(bias_i[:], [[1, NBINS]], channel_multiplier=0)
            bias_f = pool.tile([P, NBINS], F32, tag="bias_f")
            nc.vector.tensor_scalar(bias_f[:], bias_i[:], -1.0, None, AluOp.mult)

            strip = pool.tile([P, N_COLS], F32, tag="strip")

            trash_a = pool.tile([P, CHUNK], BF16, tag="trash_a")
            trash_d = pool.tile([P, CHUNK], BF16, tag="trash_d")

            for chunk in range(NCHUNK):
                sl = bass.ts(chunk, CHUNK)
                # load logits + targets for this chunk
                x0 = pool.tile([P, CHUNK], F32, tag="x0", bufs=2)
                x1 = pool.tile([P, CHUNK], F32, tag="x1", bufs=2)
                x2 = pool.tile([P, CHUNK], F32, tag="x2", bufs=2)
                t1 = pool.tile([P, CHUNK], I32, tag="t1", bufs=1)
                t2 = pool.tile([P, CHUNK], I32, tag="t2", bufs=1)
                nc.sync.dma_start(x0[:], pred[0, :, sl])
                nc.sync.dma_start(x1[:], pred[1, :, sl])
                nc.sync.dma_start(x2[:], pred[2, :, sl])
                nc.sync.dma_start(t1[:], targ[0, :, sl])
                nc.sync.dma_start(t2[:], targ[1, :, sl])

                # softmax over the 3 channels; only p1, p2 needed
                e0 = pool.tile([P, CHUNK], F32, tag="e0", bufs=1)
                e1 = pool.tile([P, CHUNK], F32, tag="e1", bufs=1)
                e2 = pool.tile([P, CHUNK], F32, tag="e2", bufs=1)
                nc.scalar.activation(e0[:], x0[:], ACT.Exp)
                nc.scalar.activation(e1[:], x1[:], ACT.Exp)
                nc.scalar.activation(e2[:], x2[:], ACT.Exp)
                # s = e0 + e1 + e2   (accumulate into e0)
                nc.vector.tensor_tensor(e0[:], e0[:], e1[:], AluOp.add)
                nc.vector.tensor_tensor(e0[:], e0[:], e2[:], AluOp.add)
                # r = 1/s  (into x0, which is dead)
                nc.vector.reciprocal_approx_fast(x0[:], e0[:])
                # p1 = e1 * r, p2 = e2 * r (in place)
                nc.vector.tensor_tensor(e1[:], e1[:], x0[:], AluOp.mult)
                nc.vector.tensor_tensor(e2[:], e2[:], x0[:], AluOp.mult)
                # packed x = t + p (into x1/x2, which are dead)
                nc.vector.scalar_tensor_tensor(
                    x1[:], t1[:], 0.0, e1[:], AluOp.add, AluOp.add)
                nc.vector.scalar_tensor_tensor(
                    x2[:], t2[:], 0.0, e2[:], AluOp.add, AluOp.add)

                for ch, xc in ((0, x1), (1, x2)):
                    # ACT value passes: G_m = sum relu(x - m)
                    for m in VAL_ACT:
                        nc.scalar.activation(
                            trash_a[:], xc[:], ACT.Relu,
                            bias=bias_f[:, m:m + 1], scale=1.0,
                            accum_out=strip[:, _val_col(chunk, ch, m):
                                            _val_col(chunk, ch, m) + 1])
                    # DVE count passes: N_{>=m} = sum [x >= m]
                    for m in CNT_DVE:
                        nc.vector.tensor_scalar(
                            trash_d[:], xc[:], float(m), 0.0,
                            AluOp.is_ge, AluOp.add,
                            accum_out=strip[:, _cnt_col(chunk, ch, m):
                                            _cnt_col(chunk, ch, m) + 1])
                    # ACT count passes: S_m = sum sign(x - m) = 2N - n
                    for m in CNT_ACT:
                        nc.scalar.activation(
                            trash_a[:], xc[:], ACT.Sign,
                            bias=bias_f[:, m:m + 1], scale=1.0,
                            accum_out=strip[:, _cnt_col(chunk, ch, m):
                                            _cnt_col(chunk, ch, m) + 1])

            nc.sync.dma_start(out[:], strip[:])

    nc.compile()
    nc.m = get_hw_module(nc.m)
    return nc


_NC_CACHE = None


def _get_nc():
    global _NC_CACHE
    if _NC_CACHE is None:
        _NC_CACHE = build_nc()
    return _NC_CACHE


def make_in_maps(predictions, targets):
    in_maps = []
    for k in range(N_CORES):
        b = k // CORES_PER_BATCH
        d0 = (k % CORES_PER_BATCH) * D_SH
        pr = np.ascontiguousarray(
            predictions[b, :, d0:d0 + D_SH]).reshape(C, P, F)
        tg = np.ascontiguousarray(
            targets[b, 1:, d0:d0 + D_SH]).reshape(2, P, F)
        in_maps.append({"pred": pr, "targ": tg})
    return in_maps


def decode(strips):
    """strips: list of N_CORES arrays [P, N_COLS] -> final scalar (f64)."""
    n_chunk_elems = float(P * CHUNK)
    # accumulate per (b, ch): raw value sums V_m and counts N_{>=m}
    Bv = np.zeros((B, 2, NBINS))       # B_m, m = 0..32
    Ng = np.zeros((B, 2, NBINS + 1))   # N_{>=m}, m = 1..33 (33 stays 0)
    Graw = np.zeros((B, 2, NBINS))
    for k in range(N_CORES):
        b = k // CORES_PER_BATCH
        s = strips[k].astype(np.float64).sum(axis=0)   # [N_COLS]
        for chunk in range(NCHUNK):
            for ch in range(2):
                for m in range(NBINS):
                    Graw[b, ch, m] += s[_val_col(chunk, ch, m)]
                for m in CNT_DVE:
                    Ng[b, ch, m - 1] += s[_cnt_col(chunk, ch, m)]
                for m in CNT_ACT:
                    sign_sum = s[_cnt_col(chunk, ch, m)]
                    Ng[b, ch, m - 1] += 0.5 * (sign_sum + n_chunk_elems)
    # G_m = B_m + sum_{i>m} N_{>=i}  ->  B_m = G_m - suffix
    for b in range(B):
        for ch in range(2):
            for m in range(NBINS):
                # sum_{i>m} N_{>=i}: Ng index i-1 over i = m+1..33
                suffix_m = Ng[b, ch, m:NBINS].sum()
                Bv[b, ch, m] = Graw[b, ch, m] - suffix_m
    # P_m = B_m - B_{m+1};  C_m = N_{>=m} - N_{>=m+1}
    Pm = np.concatenate([Bv[:, :, :-1] - Bv[:, :, 1:], Bv[:, :, -1:]], axis=2)
    Cm = Ng[:, :, :NBINS - 1] - Ng[:, :, 1:NBINS]    # m = 1..32

    s_bg = Pm[:, :, 0:1]
    s_i = Pm[:, :, 1:]
    n_i = Cm
    dice = 1.0 - (2.0 * s_i + EPS) / (s_bg + s_i + n_i + EPS)
    present = (n_i > 0.5).astype(np.float64)
    per_class = (dice * present).sum(axis=(0, 2)) / np.maximum(
        present.sum(axis=(0, 2)), 1.0)
    return per_class.mean()


def kernel(predictions, targets):
    predictions = np.asarray(predictions, dtype=np.float32)
    targets = np.asarray(targets, dtype=np.int32)
    nc = _get_nc()
    in_maps = make_in_maps(predictions, targets)
    res = bass_utils.run_bass_kernel_spmd(
        nc, in_maps, core_ids=list(range(N_CORES)))
    strips = [res.results[k]["out"] for k in range(N_CORES)]
    return np.float32(decode(strips))


# revision 5
# speedup vs baseline: 1.0058x; 1.0058x over previous
"""BatchBlobLoss Trainium2 kernel (8-core SPMD).

Reference computation:
  p = softmax(predictions, axis=1)[:, 1:]          # foreground class probs
  per (b, c): segment-sum of p keyed by instance id t = targets[b, c]
  soft-dice per (b, c, instance), masked mean -> scalar.

Device strategy (per core; cores k = 0..7 get batch b = k//4 and
D-slices 16*(k%4) .. +16):
  The 33-bin segment sum is computed with one fused reduce-op per bin:
    x = t + p  (p in (0,1) strictly, so floor(x) = t)
    ACT (scalar engine):  G_m  = sum relu(x - m)       = B_m + sum_{i>m} N_{>=i}
    ACT (Sign):           S_m  = sum sign(x - m)       = 2*N_{>=m} - n
    DVE (is_ge + accum):  N_{>=m} = sum [x >= m]
  where B_m = sum_{t >= m} p. Host (float64) recovers
    P_m = B_m - B_{m+1}  (per-instance prob sums) and C_m = N_{>=m} - N_{>=m+1}
  and evaluates the tiny dice formula. Per-partition accumulator strips
  [128, n_cols] are DMA'd out and reduced on host.
"""
import numpy as np
from contextlib import ExitStack

import concourse.bass as bass
import concourse.tile as tile
from concourse import bacc, mybir
from concourse import bass_utils
from concourse.bass_interp import get_hw_module

# problem shape (hardcoded per contest rules)
B, C, D, H, W = 2, 3, 64, 256, 256
M = 32
EPS = 1e-5
N_CORES = 8
CORES_PER_BATCH = 4
D_SH = D // CORES_PER_BATCH      # 16 depth slices per core
P = 128
NVOX = D_SH * H * W              # 1,048,576 voxels per core per channel
F = NVOX // P                    # 8192
CHUNK = 4096
NCHUNK = F // CHUNK              # 2
NBINS = 33                       # ids 0..32

# engine split for the 65 binning passes per (chunk, channel)
VAL_ACT = list(range(NBINS))     # value bins via ACT Relu -> G_m
CNT_DVE = list(range(1, 28))     # count bins via DVE is_ge -> N_{>=m}
CNT_ACT = list(range(28, NBINS)) # count bins via ACT Sign -> 2N - n

COLS_PER_SET = 65                # 33 value + 32 count columns
N_COLS = NCHUNK * 2 * COLS_PER_SET

F32 = mybir.dt.float32
BF16 = mybir.dt.bfloat16
I32 = mybir.dt.int32


def _val_col(chunk, ch, m):
    return (chunk * 2 + ch) * COLS_PER_SET + m


def _cnt_col(chunk, ch, m):
    return (chunk * 2 + ch) * COLS_PER_SET + NBINS + (m - 1)


def build_nc(scopes=False):
    AluOp = mybir.AluOpType
    ACT = mybir.ActivationFunctionType

    import contextlib

    def sc(nc, name):
        return nc.named_scope(name) if scopes else contextlib.nullcontext()

    nc = bacc.Bacc("TRN2", target_bir_lowering=False, debug=False,
                   num_devices=N_CORES)
    pred = nc.dram_tensor("pred", [C, P, F], F32, kind="ExternalInput").ap()
    targ = nc.dram_tensor("targ", [2, P, F], I32, kind="ExternalInput").ap()
    out = nc.dram_tensor("out", [P, N_COLS], F32, kind="ExternalOutput").ap()

    with tile.TileContext(nc) as tc:
        with ExitStack() as ctx:
            pool = ctx.enter_context(tc.tile_pool(name="main", bufs=1))

            # bias strip: column m holds -m (f32), for ACT bias
            bias_i = pool.tile([P, NBINS], I32, tag="bias_i")
            nc.gpsimd.iota(bias_i[:], [[1, NBINS]], channel_multiplier=0)
            bias_f = pool.tile([P, NBINS], F32, tag="bias_f")
            nc.vector.tensor_scalar(bias_f[:], bias_i[:], -1.0, None, AluOp.mult)

            strip = pool.tile([P, N_COLS], F32, tag="strip")

            trash_a = pool.tile([P, CHUNK], BF16, tag="trash_a")
            trash_d = pool.tile([P, CHUNK], BF16, tag="trash_d")

            for chunk in range(NCHUNK):
                sl = bass.ts(chunk, CHUNK)
                # load logits + targets for this chunk
                x0 = pool.tile([P, CHUNK], F32, tag="x0", bufs=2)
                x1 = pool.tile([P, CHUNK], F32, tag="x1", bufs=2)
                x2 = pool.tile([P, CHUNK], F32, tag="x2", bufs=2)
                t1 = pool.tile([P, CHUNK], I32, tag="t1", bufs=1)
                t2 = pool.tile([P, CHUNK], I32, tag="t2", bufs=1)
                nc.sync.dma_start(x0[:], pred[0, :, sl])
                nc.sync.dma_start(x1[:], pred[1, :, sl])
                nc.sync.dma_start(x2[:], pred[2, :, sl])
                nc.sync.dma_start(t1[:], targ[0, :, sl])
                nc.sync.dma_start(t2[:], targ[1, :, sl])

                # softmax over the 3 channels; only p1, p2 needed
                e0 = pool.tile([P, CHUNK], F32, tag="e0", bufs=1)
                e1 = pool.tile([P, CHUNK], F32, tag="e1", bufs=1)
                e2 = pool.tile([P, CHUNK], F32, tag="e2", bufs=1)
                with sc(nc, f"prep_exp{chunk}"):
                    nc.scalar.activation(e0[:], x0[:], ACT.Exp)
                    nc.scalar.activation(e1[:], x1[:], ACT.Exp)
                    nc.scalar.activation(e2[:], x2[:], ACT.Exp)
                with sc(nc, f"prep_dve{chunk}"):
                    # s = e0 + e1 + e2   (accumulate into e0)
                    nc.vector.tensor_tensor(e0[:], e0[:], e1[:], AluOp.add)
                    nc.vector.tensor_tensor(e0[:], e0[:], e2[:], AluOp.add)
                    # r = 1/s  (into x0, which is dead)
                    nc.vector.reciprocal_approx_fast(x0[:], e0[:])
                    # p1 = e1 * r, p2 = e2 * r (in place)
                    nc.vector.tensor_tensor(e1[:], e1[:], x0[:], AluOp.mult)
                    nc.vector.tensor_tensor(e2[:], e2[:], x0[:], AluOp.mult)
                    # packed x = t + p (into x1/x2, which are dead)
                    nc.vector.scalar_tensor_tensor(
                        x1[:], t1[:], 0.0, e1[:], AluOp.add, AluOp.add)
                    nc.vector.scalar_tensor_tensor(
                        x2[:], t2[:], 0.0, e2[:], AluOp.add, AluOp.add)

                for ch, xc in ((0, x1), (1, x2)):
                    # ACT value passes: G_m = sum relu(x - m)
                    with sc(nc, f"bin{chunk}_{ch}_act_v"):
                        for m in VAL_ACT:
                            nc.scalar.activation(
                                trash_a[:], xc[:], ACT.Relu,
                                bias=bias_f[:, m:m + 1], scale=1.0,
                                accum_out=strip[:, _val_col(chunk, ch, m):
                                                _val_col(chunk, ch, m) + 1])
                    # DVE count passes: N_{>=m} = sum [x >= m]
                    with sc(nc, f"bin{chunk}_{ch}_dve_c"):
                        for m in CNT_DVE:
                            nc.vector.tensor_scalar(
                                trash_d[:], xc[:], float(m), 0.0,
                                AluOp.is_ge, AluOp.add,
                                accum_out=strip[:, _cnt_col(chunk, ch, m):
                                                _cnt_col(chunk, ch, m) + 1])
                    # ACT count passes: S_m = sum sign(x - m) = 2N - n
                    with sc(nc, f"bin{chunk}_{ch}_act_c"):
                        for m in CNT_ACT:
                            nc.scalar.activation(
                                trash_a[:], xc[:], ACT.Sign,
                                bias=bias_f[:, m:m + 1], scale=1.0,
                                accum_out=strip[:, _cnt_col(chunk, ch, m):
                                                _cnt_col(chunk, ch, m) + 1])

            nc.sync.dma_start(out[:], strip[:])

    nc.compile()
    nc.m = get_hw_module(nc.m)
    return nc


_NC_CACHE = None


def _get_nc():
    global _NC_CACHE
    if _NC_CACHE is None:
        _NC_CACHE = build_nc()
    return _NC_CACHE


def make_in_maps(predictions, targets):
    in_maps = []
    for k in range(N_CORES):
        b = k // CORES_PER_BATCH
        d0 = (k % CORES_PER_BATCH) * D_SH
        pr = np.ascontiguousarray(
            predictions[b, :, d0:d0 + D_SH]).reshape(C, P, F)
        tg = np.ascontiguousarray(
            targets[b, 1:, d0:d0 + D_SH]).reshape(2, P, F)
        in_maps.append({"pred": pr, "targ": tg})
    return in_maps


def decode(strips):
    """strips: list of N_CORES arrays [P, N_COLS] -> final scalar (f64)."""
    n_chunk_elems = float(P * CHUNK)
    # accumulate per (b, ch): raw value sums V_m and counts N_{>=m}
    Bv = np.zeros((B, 2, NBINS))       # B_m, m = 0..32
    Ng = np.zeros((B, 2, NBINS + 1))   # N_{>=m}, m = 1..33 (33 stays 0)
    Graw = np.zeros((B, 2, NBINS))
    for k in range(N_CORES):
        b = k // CORES_PER_BATCH
        s = strips[k].astype(np.float64).sum(axis=0)   # [N_COLS]
        for chunk in range(NCHUNK):
            for ch in range(2):
                for m in range(NBINS):
                    Graw[b, ch, m] += s[_val_col(chunk, ch, m)]
                for m in CNT_DVE:
                    Ng[b, ch, m - 1] += s[_cnt_col(chunk, ch, m)]
                for m in CNT_ACT:
                    sign_sum = s[_cnt_col(chunk, ch, m)]
                    Ng[b, ch, m - 1] += 0.5 * (sign_sum + n_chunk_elems)
    # G_m = B_m + sum_{i>m} N_{>=i}  ->  B_m = G_m - suffix
    for b in range(B):
        for ch in range(2):
            for m in range(NBINS):
                # sum_{i>m} N_{>=i}: Ng index i-1 over i = m+1..33
                suffix_m = Ng[b, ch, m:NBINS].sum()
                Bv[b, ch, m] = Graw[b, ch, m] - suffix_m
    # P_m = B_m - B_{m+1};  C_m = N_{>=m} - N_{>=m+1}
    Pm = np.concatenate([Bv[:, :, :-1] - Bv[:, :, 1:], Bv[:, :, -1:]], axis=2)
    Cm = Ng[:, :, :NBINS - 1] - Ng[:, :, 1:NBINS]    # m = 1..32

    s_bg = Pm[:, :, 0:1]
    s_i = Pm[:, :, 1:]
    n_i = Cm
    dice = 1.0 - (2.0 * s_i + EPS) / (s_bg + s_i + n_i + EPS)
    present = (n_i > 0.5).astype(np.float64)
    per_class = (dice * present).sum(axis=(0, 2)) / np.maximum(
        present.sum(axis=(0, 2)), 1.0)
    return per_class.mean()


def kernel(predictions, targets):
    predictions = np.asarray(predictions, dtype=np.float32)
    targets = np.asarray(targets, dtype=np.int32)
    nc = _get_nc()
    in_maps = make_in_maps(predictions, targets)
    res = bass_utils.run_bass_kernel_spmd(
        nc, in_maps, core_ids=list(range(N_CORES)))
    strips = [res.results[k]["out"] for k in range(N_CORES)]
    return np.float32(decode(strips))


# revision 6
# speedup vs baseline: 1.0569x; 1.0508x over previous
"""BatchBlobLoss Trainium2 kernel (8-core SPMD).

Reference computation:
  p = softmax(predictions, axis=1)[:, 1:]          # foreground class probs
  per (b, c): segment-sum of p keyed by instance id t = targets[b, c]
  soft-dice per (b, c, instance), masked mean -> scalar.

Device strategy (per core; cores k = 0..7 get batch b = k//4 and
D-slices 16*(k%4) .. +16):
  The 33-bin segment sum is computed with one fused reduce-op per bin:
    x = t + p  (p in (0,1) strictly, so floor(x) = t)
    ACT (scalar engine):  G_m  = sum relu(x - m)       = B_m + sum_{i>m} N_{>=i}
    ACT (Sign):           S_m  = sum sign(x - m)       = 2*N_{>=m} - n
    DVE (is_ge + accum):  N_{>=m} = sum [x >= m]
  where B_m = sum_{t >= m} p. Host (float64) recovers
    P_m = B_m - B_{m+1}  (per-instance prob sums) and C_m = N_{>=m} - N_{>=m+1}
  and evaluates the tiny dice formula. Per-partition accumulator strips
  [128, n_cols] are DMA'd out and reduced on host.
"""
import numpy as np
from contextlib import ExitStack

import concourse.bass as bass
import concourse.tile as tile
from concourse import bacc, mybir
from concourse import bass_utils
from concourse.bass_interp import get_hw_module

# problem shape (hardcoded per contest rules)
B, C, D, H, W = 2, 3, 64, 256, 256
M = 32
EPS = 1e-5
N_CORES = 8
CORES_PER_BATCH = 4
D_SH = D // CORES_PER_BATCH      # 16 depth slices per core
P = 128
NVOX = D_SH * H * W              # 1,048,576 voxels per core per channel
F = NVOX // P                    # 8192
CHUNK = 4096
NCHUNK = F // CHUNK              # 2
NBINS = 33                       # ids 0..32

# engine split for the 65 binning passes per channel (full 8192-rows)
VAL_ACT = list(range(NBINS))      # value bins via ACT Relu -> G_m
VAL_DVE = []                      # value bins via DVE (sub,max) -> G_m
CNT_DVE = list(range(1, 30))      # count bins via DVE is_ge -> N_{>=m}
CNT_ACT = list(range(30, NBINS))  # count bins via ACT Sign -> 2N - n

COLS_PER_SET = 65                 # 33 value + 32 count columns
N_COLS = 2 * COLS_PER_SET

F32 = mybir.dt.float32
BF16 = mybir.dt.bfloat16
I32 = mybir.dt.int32


def _val_col(ch, m):
    return ch * COLS_PER_SET + m


def _cnt_col(ch, m):
    return ch * COLS_PER_SET + NBINS + (m - 1)


def build_nc(scopes=False):
    AluOp = mybir.AluOpType
    ACT = mybir.ActivationFunctionType

    import contextlib

    def sc(nc, name):
        return nc.named_scope(name) if scopes else contextlib.nullcontext()

    nc = bacc.Bacc("TRN2", target_bir_lowering=False, debug=False,
                   num_devices=N_CORES)
    pred = nc.dram_tensor("pred", [C, P, F], F32, kind="ExternalInput").ap()
    targ = nc.dram_tensor("targ", [2, P, F], I32, kind="ExternalInput").ap()
    out = nc.dram_tensor("out", [P, N_COLS], F32, kind="ExternalOutput").ap()

    with tile.TileContext(nc) as tc:
        with ExitStack() as ctx:
            pool = ctx.enter_context(tc.tile_pool(name="main", bufs=1))

            # bias strip: column m holds -m (f32), for ACT bias
            bias_i = pool.tile([P, NBINS], I32, tag="bias_i")
            nc.gpsimd.iota(bias_i[:], [[1, NBINS]], channel_multiplier=0)
            bias_f = pool.tile([P, NBINS], F32, tag="bias_f")
            nc.vector.tensor_scalar(bias_f[:], bias_i[:], -1.0, None, AluOp.mult)

            strip = pool.tile([P, N_COLS], F32, tag="strip")

            trash_a = pool.tile([P, F], BF16, tag="trash_a")
            trash_d = pool.tile([P, F], BF16, tag="trash_d")

            xp1 = pool.tile([P, F], F32, tag="xp1")
            xp2 = pool.tile([P, F], F32, tag="xp2")

            for chunk in range(NCHUNK):
                sl = bass.ts(chunk, CHUNK)
                # load logits + targets for this chunk
                x0 = pool.tile([P, CHUNK], F32, tag="x0", bufs=1)
                x1 = pool.tile([P, CHUNK], F32, tag="x1", bufs=1)
                x2 = pool.tile([P, CHUNK], F32, tag="x2", bufs=1)
                t1 = pool.tile([P, CHUNK], I32, tag="t1", bufs=1)
                t2 = pool.tile([P, CHUNK], I32, tag="t2", bufs=1)
                scr = pool.tile([P, CHUNK], F32, tag="scr", bufs=1)
                nc.sync.dma_start(x0[:], pred[0, :, sl])
                nc.sync.dma_start(x1[:], pred[1, :, sl])
                nc.sync.dma_start(x2[:], pred[2, :, sl])
                nc.sync.dma_start(t1[:], targ[0, :, sl])
                nc.sync.dma_start(t2[:], targ[1, :, sl])

                with sc(nc, f"prep_exp{chunk}"):
                    # in-place exp: x_c <- e_c
                    nc.scalar.activation(x0[:], x0[:], ACT.Exp)
                    nc.scalar.activation(x1[:], x1[:], ACT.Exp)
                    nc.scalar.activation(x2[:], x2[:], ACT.Exp)
                with sc(nc, f"prep_dve{chunk}"):
                    # s = e0 + e1 + e2 into scr
                    nc.vector.tensor_tensor(scr[:], x0[:], x1[:], AluOp.add)
                    nc.vector.tensor_tensor(scr[:], scr[:], x2[:], AluOp.add)
                    # r = 1/s into x0 (e0 dead)
                    nc.vector.reciprocal_approx_fast(x0[:], scr[:])
                    # p1, p2 in place
                    nc.vector.tensor_tensor(x1[:], x1[:], x0[:], AluOp.mult)
                    nc.vector.tensor_tensor(x2[:], x2[:], x0[:], AluOp.mult)
                    # packed x = t + p into the full-row tiles
                    nc.vector.scalar_tensor_tensor(
                        xp1[:, sl], t1[:], 0.0, x1[:], AluOp.add, AluOp.add)
                    nc.vector.scalar_tensor_tensor(
                        xp2[:, sl], t2[:], 0.0, x2[:], AluOp.add, AluOp.add)

            # binning over the full [P, F] packed rows
            # ACT: all Relu ops first (both channels), then all Sign ops --
            # minimizes activation-table switches.
            with sc(nc, "bin_act_v"):
                for ch, xc in ((0, xp1), (1, xp2)):
                    for m in VAL_ACT:
                        nc.scalar.activation(
                            trash_a[:], xc[:], ACT.Relu,
                            bias=bias_f[:, m:m + 1], scale=1.0,
                            accum_out=strip[:, _val_col(ch, m):
                                            _val_col(ch, m) + 1])
            with sc(nc, "bin_act_c"):
                for ch, xc in ((0, xp1), (1, xp2)):
                    for m in CNT_ACT:
                        nc.scalar.activation(
                            trash_a[:], xc[:], ACT.Sign,
                            bias=bias_f[:, m:m + 1], scale=1.0,
                            accum_out=strip[:, _cnt_col(ch, m):
                                            _cnt_col(ch, m) + 1])
            with sc(nc, "bin_dve"):
                for ch, xc in ((0, xp1), (1, xp2)):
                    for m in VAL_DVE:
                        nc.vector.tensor_scalar(
                            trash_d[:], xc[:], float(m), 0.0,
                            AluOp.subtract, AluOp.max,
                            accum_out=strip[:, _val_col(ch, m):
                                            _val_col(ch, m) + 1])
                    for m in CNT_DVE:
                        nc.vector.tensor_scalar(
                            trash_d[:], xc[:], float(m), 0.0,
                            AluOp.is_ge, AluOp.add,
                            accum_out=strip[:, _cnt_col(ch, m):
                                            _cnt_col(ch, m) + 1])

            nc.sync.dma_start(out[:], strip[:])

    nc.compile()
    nc.m = get_hw_module(nc.m)
    return nc


_NC_CACHE = None


def _get_nc():
    global _NC_CACHE
    if _NC_CACHE is None:
        _NC_CACHE = build_nc()
    return _NC_CACHE


def make_in_maps(predictions, targets):
    in_maps = []
    for k in range(N_CORES):
        b = k // CORES_PER_BATCH
        d0 = (k % CORES_PER_BATCH) * D_SH
        pr = np.ascontiguousarray(
            predictions[b, :, d0:d0 + D_SH]).reshape(C, P, F)
        tg = np.ascontiguousarray(
            targets[b, 1:, d0:d0 + D_SH]).reshape(2, P, F)
        in_maps.append({"pred": pr, "targ": tg})
    return in_maps


def decode(strips):
    """strips: list of N_CORES arrays [P, N_COLS] -> final scalar (f64)."""
    n_row_elems = float(P * F)
    Bv = np.zeros((B, 2, NBINS))       # B_m, m = 0..32
    Ng = np.zeros((B, 2, NBINS + 1))   # N_{>=m}, m = 1..33 (33 stays 0)
    Graw = np.zeros((B, 2, NBINS))
    for k in range(N_CORES):
        b = k // CORES_PER_BATCH
        s = strips[k].astype(np.float64).sum(axis=0)   # [N_COLS]
        for ch in range(2):
            for m in range(NBINS):
                Graw[b, ch, m] += s[_val_col(ch, m)]
            for m in CNT_DVE:
                Ng[b, ch, m - 1] += s[_cnt_col(ch, m)]
            for m in CNT_ACT:
                sign_sum = s[_cnt_col(ch, m)]
                Ng[b, ch, m - 1] += 0.5 * (sign_sum + n_row_elems)
    # G_m = B_m + sum_{i>m} N_{>=i}  ->  B_m = G_m - suffix
    for b in range(B):
        for ch in range(2):
            for m in range(NBINS):
                # sum_{i>m} N_{>=i}: Ng index i-1 over i = m+1..33
                suffix_m = Ng[b, ch, m:NBINS].sum()
                Bv[b, ch, m] = Graw[b, ch, m] - suffix_m
    # P_m = B_m - B_{m+1};  C_m = N_{>=m} - N_{>=m+1}
    Pm = np.concatenate([Bv[:, :, :-1] - Bv[:, :, 1:], Bv[:, :, -1:]], axis=2)
    Cm = Ng[:, :, :NBINS - 1] - Ng[:, :, 1:NBINS]    # m = 1..32

    s_bg = Pm[:, :, 0:1]
    s_i = Pm[:, :, 1:]
    n_i = Cm
    dice = 1.0 - (2.0 * s_i + EPS) / (s_bg + s_i + n_i + EPS)
    present = (n_i > 0.5).astype(np.float64)
    per_class = (dice * present).sum(axis=(0, 2)) / np.maximum(
        present.sum(axis=(0, 2)), 1.0)
    return per_class.mean()


def kernel(predictions, targets):
    predictions = np.asarray(predictions, dtype=np.float32)
    targets = np.asarray(targets, dtype=np.int32)
    nc = _get_nc()
    in_maps = make_in_maps(predictions, targets)
    res = bass_utils.run_bass_kernel_spmd(
        nc, in_maps, core_ids=list(range(N_CORES)))
    strips = [res.results[k]["out"] for k in range(N_CORES)]
    return np.float32(decode(strips))


# revision 7
# speedup vs baseline: 1.0872x; 1.0286x over previous
"""BatchBlobLoss Trainium2 kernel (8-core SPMD).

Reference computation:
  p = softmax(predictions, axis=1)[:, 1:]          # foreground class probs
  per (b, c): segment-sum of p keyed by instance id t = targets[b, c]
  soft-dice per (b, c, instance), masked mean -> scalar.

Device strategy (per core; cores k = 0..7 get batch b = k//4 and
D-slices 16*(k%4) .. +16):
  The 33-bin segment sum is computed with one fused reduce-op per bin:
    x = t + p  (p in (0,1) strictly, so floor(x) = t)
    ACT (scalar engine):  G_m  = sum relu(x - m)       = B_m + sum_{i>m} N_{>=i}
    ACT (Sign):           S_m  = sum sign(x - m)       = 2*N_{>=m} - n
    DVE (is_ge + accum):  N_{>=m} = sum [x >= m]
  where B_m = sum_{t >= m} p. Host (float64) recovers
    P_m = B_m - B_{m+1}  (per-instance prob sums) and C_m = N_{>=m} - N_{>=m+1}
  and evaluates the tiny dice formula. Per-partition accumulator strips
  [128, n_cols] are DMA'd out and reduced on host.
"""
import numpy as np
from contextlib import ExitStack

import concourse.bass as bass
import concourse.tile as tile
from concourse import bacc, mybir
from concourse import bass_utils
from concourse.bass_interp import get_hw_module

# problem shape (hardcoded per contest rules)
B, C, D, H, W = 2, 3, 64, 256, 256
M = 32
EPS = 1e-5
N_CORES = 8
CORES_PER_BATCH = 4
D_SH = D // CORES_PER_BATCH      # 16 depth slices per core
P = 128
NVOX = D_SH * H * W              # 1,048,576 voxels per core per channel
F = NVOX // P                    # 8192
CHUNK = 4096
NCHUNK = F // CHUNK              # 2
NBINS = 33                       # ids 0..32

# engine split for the 65 binning passes per channel (full 8192-rows)
VAL_SPLIT = list(range(0, 6))     # value bins binned per-chunk (overlap prep)
VAL_ACT = list(range(6, NBINS))   # value bins via ACT Relu -> G_m (full row)
VAL_DVE = []                      # value bins via DVE (sub,max) -> G_m
CNT_DVE = list(range(1, 30))      # count bins via DVE is_ge -> N_{>=m}
CNT_ACT = list(range(30, NBINS))  # count bins via ACT Sign -> 2N - n

COLS_PER_SET = 65                 # 33 value + 32 count columns
N_SPLIT_COLS = 2 * len(VAL_SPLIT)  # second-chunk columns for split bins
N_COLS = 2 * COLS_PER_SET + N_SPLIT_COLS

F32 = mybir.dt.float32
BF16 = mybir.dt.bfloat16
I32 = mybir.dt.int32


def _val_col(ch, m):
    return ch * COLS_PER_SET + m


def _cnt_col(ch, m):
    return ch * COLS_PER_SET + NBINS + (m - 1)


def _split_col(ch, i):
    # chunk-1 partial for VAL_SPLIT[i]; chunk-0 partial lives in _val_col
    return 2 * COLS_PER_SET + ch * len(VAL_SPLIT) + i


def build_nc(scopes=False):
    AluOp = mybir.AluOpType
    ACT = mybir.ActivationFunctionType

    import contextlib

    def sc(nc, name):
        return nc.named_scope(name) if scopes else contextlib.nullcontext()

    nc = bacc.Bacc("TRN2", target_bir_lowering=False, debug=False,
                   num_devices=N_CORES)
    pred = nc.dram_tensor("pred", [C, P, F], F32, kind="ExternalInput").ap()
    targ = nc.dram_tensor("targ", [2, P, F], I32, kind="ExternalInput").ap()
    out = nc.dram_tensor("out", [P, N_COLS], F32, kind="ExternalOutput").ap()

    with tile.TileContext(nc) as tc:
        with ExitStack() as ctx:
            pool = ctx.enter_context(tc.tile_pool(name="main", bufs=1))

            # bias strip: column m holds -m (f32), for ACT bias
            bias_i = pool.tile([P, NBINS], I32, tag="bias_i")
            nc.gpsimd.iota(bias_i[:], [[1, NBINS]], channel_multiplier=0)
            bias_f = pool.tile([P, NBINS], F32, tag="bias_f")
            nc.vector.tensor_scalar(bias_f[:], bias_i[:], -1.0, None, AluOp.mult)

            strip = pool.tile([P, N_COLS], F32, tag="strip")

            trash_a = pool.tile([P, F], BF16, tag="trash_a")
            trash_d = pool.tile([P, F], BF16, tag="trash_d")

            xp1 = pool.tile([P, F], F32, tag="xp1")
            xp2 = pool.tile([P, F], F32, tag="xp2")

            for chunk in range(NCHUNK):
                sl = bass.ts(chunk, CHUNK)
                # load logits + targets for this chunk
                x0 = pool.tile([P, CHUNK], F32, tag="x0", bufs=1)
                x1 = pool.tile([P, CHUNK], F32, tag="x1", bufs=1)
                x2 = pool.tile([P, CHUNK], F32, tag="x2", bufs=1)
                t1 = pool.tile([P, CHUNK], I32, tag="t1", bufs=1)
                t2 = pool.tile([P, CHUNK], I32, tag="t2", bufs=1)
                scr = pool.tile([P, CHUNK], F32, tag="scr", bufs=1)
                nc.sync.dma_start(x0[:], pred[0, :, sl])
                nc.sync.dma_start(x1[:], pred[1, :, sl])
                nc.sync.dma_start(x2[:], pred[2, :, sl])
                nc.sync.dma_start(t1[:], targ[0, :, sl])
                nc.sync.dma_start(t2[:], targ[1, :, sl])

                with sc(nc, f"prep_exp{chunk}"):
                    # in-place exp: x_c <- e_c
                    nc.scalar.activation(x0[:], x0[:], ACT.Exp)
                    nc.scalar.activation(x1[:], x1[:], ACT.Exp)
                    nc.scalar.activation(x2[:], x2[:], ACT.Exp)
                with sc(nc, f"prep_dve{chunk}"):
                    # s = e0 + e1 + e2 into scr
                    nc.vector.tensor_tensor(scr[:], x0[:], x1[:], AluOp.add)
                    nc.vector.tensor_tensor(scr[:], scr[:], x2[:], AluOp.add)
                    # r = 1/s into x0 (e0 dead)
                    nc.vector.reciprocal_approx_fast(x0[:], scr[:])
                    # p1, p2 in place
                    nc.vector.tensor_tensor(x1[:], x1[:], x0[:], AluOp.mult)
                    nc.vector.tensor_tensor(x2[:], x2[:], x0[:], AluOp.mult)
                    # packed x = t + p into the full-row tiles
                    nc.vector.scalar_tensor_tensor(
                        xp1[:, sl], t1[:], 0.0, x1[:], AluOp.add, AluOp.add)
                    nc.vector.scalar_tensor_tensor(
                        xp2[:, sl], t2[:], 0.0, x2[:], AluOp.add, AluOp.add)

                # split value bins: bin this chunk's halves now so ACT has
                # work while the other chunk is being prepped
                with sc(nc, f"bin_split{chunk}"):
                    for ch, xpc in ((0, xp1), (1, xp2)):
                        for i, m in enumerate(VAL_SPLIT):
                            col = (_val_col(ch, m) if chunk == 0
                                   else _split_col(ch, i))
                            nc.scalar.activation(
                                trash_a[:, sl], xpc[:, sl], ACT.Relu,
                                bias=bias_f[:, m:m + 1], scale=1.0,
                                accum_out=strip[:, col:col + 1])

            # binning over the full [P, F] packed rows
            # ACT: all Relu ops first (both channels), then all Sign ops --
            # minimizes activation-table switches.
            with sc(nc, "bin_act_v"):
                for ch, xc in ((0, xp1), (1, xp2)):
                    for m in VAL_ACT:
                        nc.scalar.activation(
                            trash_a[:], xc[:], ACT.Relu,
                            bias=bias_f[:, m:m + 1], scale=1.0,
                            accum_out=strip[:, _val_col(ch, m):
                                            _val_col(ch, m) + 1])
            with sc(nc, "bin_act_c"):
                for ch, xc in ((0, xp1), (1, xp2)):
                    for m in CNT_ACT:
                        nc.scalar.activation(
                            trash_a[:], xc[:], ACT.Sign,
                            bias=bias_f[:, m:m + 1], scale=1.0,
                            accum_out=strip[:, _cnt_col(ch, m):
                                            _cnt_col(ch, m) + 1])
            with sc(nc, "bin_dve"):
                for ch, xc in ((0, xp1), (1, xp2)):
                    for m in VAL_DVE:
                        nc.vector.tensor_scalar(
                            trash_d[:], xc[:], float(m), 0.0,
                            AluOp.subtract, AluOp.max,
                            accum_out=strip[:, _val_col(ch, m):
                                            _val_col(ch, m) + 1])
                    for m in CNT_DVE:
                        nc.vector.tensor_scalar(
                            trash_d[:], xc[:], float(m), 0.0,
                            AluOp.is_ge, AluOp.add,
                            accum_out=strip[:, _cnt_col(ch, m):
                                            _cnt_col(ch, m) + 1])

            nc.sync.dma_start(out[:], strip[:])

    nc.compile()
    nc.m = get_hw_module(nc.m)
    return nc


_NC_CACHE = None


def _get_nc():
    global _NC_CACHE
    if _NC_CACHE is None:
        _NC_CACHE = build_nc()
    return _NC_CACHE


def make_in_maps(predictions, targets):
    in_maps = []
    for k in range(N_CORES):
        b = k // CORES_PER_BATCH
        d0 = (k % CORES_PER_BATCH) * D_SH
        pr = np.ascontiguousarray(
            predictions[b, :, d0:d0 + D_SH]).reshape(C, P, F)
        tg = np.ascontiguousarray(
            targets[b, 1:, d0:d0 + D_SH]).reshape(2, P, F)
        in_maps.append({"pred": pr, "targ": tg})
    return in_maps


def decode(strips):
    """strips: list of N_CORES arrays [P, N_COLS] -> final scalar (f64)."""
    n_row_elems = float(P * F)
    Bv = np.zeros((B, 2, NBINS))       # B_m, m = 0..32
    Ng = np.zeros((B, 2, NBINS + 1))   # N_{>=m}, m = 1..33 (33 stays 0)
    Graw = np.zeros((B, 2, NBINS))
    for k in range(N_CORES):
        b = k // CORES_PER_BATCH
        s = strips[k].astype(np.float64).sum(axis=0)   # [N_COLS]
        for ch in range(2):
            for m in range(NBINS):
                Graw[b, ch, m] += s[_val_col(ch, m)]
            for i, m in enumerate(VAL_SPLIT):
                Graw[b, ch, m] += s[_split_col(ch, i)]
            for m in CNT_DVE:
                Ng[b, ch, m - 1] += s[_cnt_col(ch, m)]
            for m in CNT_ACT:
                sign_sum = s[_cnt_col(ch, m)]
                Ng[b, ch, m - 1] += 0.5 * (sign_sum + n_row_elems)
    # G_m = B_m + sum_{i>m} N_{>=i}  ->  B_m = G_m - suffix
    for b in range(B):
        for ch in range(2):
            for m in range(NBINS):
                # sum_{i>m} N_{>=i}: Ng index i-1 over i = m+1..33
                suffix_m = Ng[b, ch, m:NBINS].sum()
                Bv[b, ch, m] = Graw[b, ch, m] - suffix_m
    # P_m = B_m - B_{m+1};  C_m = N_{>=m} - N_{>=m+1}
    Pm = np.concatenate([Bv[:, :, :-1] - Bv[:, :, 1:], Bv[:, :, -1:]], axis=2)
    Cm = Ng[:, :, :NBINS - 1] - Ng[:, :, 1:NBINS]    # m = 1..32

    s_bg = Pm[:, :, 0:1]
    s_i = Pm[:, :, 1:]
    n_i = Cm
    dice = 1.0 - (2.0 * s_i + EPS) / (s_bg + s_i + n_i + EPS)
    present = (n_i > 0.5).astype(np.float64)
    per_class = (dice * present).sum(axis=(0, 2)) / np.maximum(
        present.sum(axis=(0, 2)), 1.0)
    return per_class.mean()


def kernel(predictions, targets):
    predictions = np.asarray(predictions, dtype=np.float32)
    targets = np.asarray(targets, dtype=np.int32)
    nc = _get_nc()
    in_maps = make_in_maps(predictions, targets)
    res = bass_utils.run_bass_kernel_spmd(
        nc, in_maps, core_ids=list(range(N_CORES)))
    strips = [res.results[k]["out"] for k in range(N_CORES)]
    return np.float32(decode(strips))


# revision 18
# speedup vs baseline: 1.0969x; 1.0089x over previous
"""BatchBlobLoss Trainium2 kernel (8-core SPMD).

Reference computation:
  p = softmax(predictions, axis=1)[:, 1:]          # foreground class probs
  per (b, c): segment-sum of p keyed by instance id t = targets[b, c]
  soft-dice per (b, c, instance), masked mean -> scalar.

Device strategy (per core; cores k = 0..7 get batch b = k//4 and
D-slices 16*(k%4) .. +16):
  The 33-bin segment sum is computed with one fused reduce-op per bin:
    x = t + p  (p in (0,1) strictly, so floor(x) = t)
    ACT (scalar engine):  G_m  = sum relu(x - m)       = B_m + sum_{i>m} N_{>=i}
    ACT (Sign):           S_m  = sum sign(x - m)       = 2*N_{>=m} - n
    DVE (is_ge + accum):  N_{>=m} = sum [x >= m]
  where B_m = sum_{t >= m} p. Host (float64) recovers
    P_m = B_m - B_{m+1}  (per-instance prob sums) and C_m = N_{>=m} - N_{>=m+1}
  and evaluates the tiny dice formula. Per-partition accumulator strips
  [128, n_cols] are DMA'd out and reduced on host.
"""
import numpy as np
from contextlib import ExitStack

import concourse.bass as bass
import concourse.tile as tile
from concourse import bacc, mybir
from concourse import bass_utils
from concourse.bass_interp import get_hw_module

# problem shape (hardcoded per contest rules)
B, C, D, H, W = 2, 3, 64, 256, 256
M = 32
EPS = 1e-5
N_CORES = 8
CORES_PER_BATCH = 4
D_SH = D // CORES_PER_BATCH      # 16 depth slices per core
P = 128
NVOX = D_SH * H * W              # 1,048,576 voxels per core per channel
F = NVOX // P                    # 8192
CHUNK = 4096
NCHUNK = F // CHUNK              # 2
NBINS = 33                       # ids 0..32

# engine split for the 65 binning passes per channel (full 8192-rows)
VAL_SPLIT = list(range(0, 6))     # value bins binned per-chunk (overlap prep)
VAL_ACT = list(range(6, NBINS))   # value bins via ACT Relu -> G_m (full row)
VAL_DVE = []                      # value bins via DVE (sub,max) -> G_m
CNT_SPLIT_DVE = [1, 2, 3]         # count bins per-chunk on raw t (DVE is_ge)
CNT_ACT = [30, 31, 32]            # count bins per-chunk on raw t (ACT Sign)
CNT_HALVES = [4, 5]               # chunk0 on DVE, chunk1 on ACT Sign
CNT_DVE = list(range(6, 30))      # count bins via DVE is_ge on x (full row)

COLS_PER_SET = 65                 # 33 value + 32 count columns
N_VSPLIT_COLS = 2 * len(VAL_SPLIT)   # second-chunk cols for split value bins
N_CSPLIT = CNT_SPLIT_DVE + CNT_ACT + CNT_HALVES  # per-chunk count bins
N_CSPLIT_COLS = 2 * len(N_CSPLIT)    # second-chunk cols for split count bins
N_COLS = 2 * COLS_PER_SET + N_VSPLIT_COLS + N_CSPLIT_COLS

F32 = mybir.dt.float32
BF16 = mybir.dt.bfloat16
I32 = mybir.dt.int32


def _val_col(ch, m):
    return ch * COLS_PER_SET + m


def _cnt_col(ch, m):
    return ch * COLS_PER_SET + NBINS + (m - 1)


def _split_col(ch, i):
    # chunk-1 partial for VAL_SPLIT[i]; chunk-0 partial lives in _val_col
    return 2 * COLS_PER_SET + ch * len(VAL_SPLIT) + i


def _csplit_col(ch, i):
    # chunk-1 partial for N_CSPLIT[i]; chunk-0 partial lives in _cnt_col
    return (2 * COLS_PER_SET + N_VSPLIT_COLS + ch * len(N_CSPLIT) + i)


def build_nc(scopes=False):
    AluOp = mybir.AluOpType
    ACT = mybir.ActivationFunctionType

    import contextlib

    def sc(nc, name):
        return nc.named_scope(name) if scopes else contextlib.nullcontext()

    nc = bacc.Bacc("TRN2", target_bir_lowering=False, debug=False,
                   num_devices=N_CORES)
    pred = nc.dram_tensor("pred", [C, P, F], F32, kind="ExternalInput").ap()
    targ = nc.dram_tensor("targ", [2, P, F], I32, kind="ExternalInput").ap()
    out = nc.dram_tensor("out", [P, N_COLS], F32, kind="ExternalOutput").ap()
    out_a = nc.dram_tensor("out_a", [P, N_COLS], F32,
                           kind="ExternalOutput").ap()

    with tile.TileContext(nc) as tc:
        with ExitStack() as ctx:
            pool = ctx.enter_context(tc.tile_pool(name="main", bufs=1))

            # bias strip: column m holds -m (f32), for ACT bias
            bias_i = pool.tile([P, NBINS], I32, tag="bias_i")
            nc.gpsimd.iota(bias_i[:], [[1, NBINS]], channel_multiplier=0)
            bias_f = pool.tile([P, NBINS], F32, tag="bias_f")
            nc.vector.tensor_scalar(bias_f[:], bias_i[:], -1.0, None, AluOp.mult)
            # half-shifted bias for Sign on raw integer t: sign(t - m + 0.5)
            bias_h = pool.tile([P, NBINS], F32, tag="bias_h")
            nc.vector.tensor_scalar(bias_h[:], bias_f[:], 0.5, None, AluOp.add)

            strip = pool.tile([P, N_COLS], F32, tag="strip")
            strip_a = pool.tile([P, N_COLS], F32, tag="strip_a")
            nc.gpsimd.memset(strip[:], 0.0)
            nc.gpsimd.memset(strip_a[:], 0.0)

            ones = pool.tile([P, 1], F32, tag="ones")
            nc.gpsimd.memset(ones[:], 1.0)

            trash_a = pool.tile([P, F], BF16, tag="trash_a")
            trash_d = pool.tile([P, F], BF16, tag="trash_d")

            xp1 = pool.tile([P, F], F32, tag="xp1")
            xp2 = pool.tile([P, F], F32, tag="xp2")

            for chunk in range(NCHUNK):
                sl = bass.ts(chunk, CHUNK)
                # load logits + targets for this chunk
                x0 = pool.tile([P, CHUNK], F32, tag="x0", bufs=1)
                x1 = pool.tile([P, CHUNK], F32, tag="x1", bufs=1)
                x2 = pool.tile([P, CHUNK], F32, tag="x2", bufs=1)
                t1 = pool.tile([P, CHUNK], I32, tag="t1", bufs=1)
                t2 = pool.tile([P, CHUNK], I32, tag="t2", bufs=1)
                scr = pool.tile([P, CHUNK], F32, tag="scr", bufs=1)
                nc.sync.dma_start(x0[:], pred[0, :, sl])
                nc.sync.dma_start(x1[:], pred[1, :, sl])
                nc.sync.dma_start(x2[:], pred[2, :, sl])
                nc.sync.dma_start(t1[:], targ[0, :, sl])
                nc.sync.dma_start(t2[:], targ[1, :, sl])

                with sc(nc, f"prep_exp{chunk}"):
                    # in-place exp: x_c <- e_c
                    nc.scalar.activation(x0[:], x0[:], ACT.Exp)
                    nc.scalar.activation(x1[:], x1[:], ACT.Exp)
                    nc.scalar.activation(x2[:], x2[:], ACT.Exp)
                with sc(nc, f"prep_dve{chunk}"):
                    # s = e0 + e1 + e2 into scr
                    nc.vector.tensor_tensor(scr[:], x0[:], x1[:], AluOp.add)
                    nc.vector.tensor_tensor(scr[:], scr[:], x2[:], AluOp.add)
                    # r = 1/s into x0 (e0 dead)
                    nc.vector.reciprocal_approx_fast(x0[:], scr[:])
                    # p1, p2 in place
                    nc.vector.tensor_tensor(x1[:], x1[:], x0[:], AluOp.mult)
                    nc.vector.tensor_tensor(x2[:], x2[:], x0[:], AluOp.mult)
                    # packed x = t + p into the full-row tiles
                    nc.vector.scalar_tensor_tensor(
                        xp1[:, sl], t1[:], 0.0, x1[:], AluOp.add, AluOp.add)
                    nc.vector.scalar_tensor_tensor(
                        xp2[:, sl], t2[:], 0.0, x2[:], AluOp.add, AluOp.add)

                # count bins on the raw int32 targets -- these only need the
                # t DMA, so they fill the engine-idle windows before/during
                # softmax prep
                with sc(nc, f"cnt_t{chunk}"):
                    for ch, tc_ in ((0, t1), (1, t2)):
                        for i, m in enumerate(CNT_SPLIT_DVE):
                            col = (_cnt_col(ch, m) if chunk == 0
                                   else _csplit_col(ch, i))
                            nc.vector.scalar_tensor_tensor(
                                trash_d[:, sl], tc_[:], float(m),
                                ones[:].to_broadcast((P, CHUNK)),
                                AluOp.is_ge, AluOp.mult,
                                accum_out=strip[:, col:col + 1])
                        for j, m in enumerate(CNT_ACT):
                            i = len(CNT_SPLIT_DVE) + j
                            col = (_cnt_col(ch, m) if chunk == 0
                                   else _csplit_col(ch, i))
                            nc.scalar.activation(
                                trash_a[:, sl], tc_[:], ACT.Sign,
                                bias=bias_h[:, m:m + 1], scale=1.0,
                                accum_out=strip_a[:, col:col + 1])

                # split value bins: bin this chunk's halves now so ACT has
                # work while the other chunk is being prepped
                with sc(nc, f"bin_split{chunk}"):
                    for ch, xpc in ((0, xp1), (1, xp2)):
                        for i, m in enumerate(VAL_SPLIT):
                            col = (_val_col(ch, m) if chunk == 0
                                   else _split_col(ch, i))
                            nc.scalar.activation(
                                trash_a[:, sl], xpc[:, sl], ACT.Relu,
                                bias=bias_f[:, m:m + 1], scale=1.0,
                                accum_out=strip_a[:, col:col + 1])

            # binning over the full [P, F] packed rows
            # ACT: all Relu ops first (both channels), then all Sign ops --
            # minimizes activation-table switches.
            with sc(nc, "bin_act_v"):
                for ch, xc in ((0, xp1), (1, xp2)):
                    for m in VAL_ACT:
                        nc.scalar.activation(
                            trash_a[:], xc[:], ACT.Relu,
                            bias=bias_f[:, m:m + 1], scale=1.0,
                            accum_out=strip_a[:, _val_col(ch, m):
                                              _val_col(ch, m) + 1])
            with sc(nc, "bin_dve"):
                for ch, xc in ((0, xp1), (1, xp2)):
                    for m in VAL_DVE:
                        nc.vector.tensor_scalar(
                            trash_d[:], xc[:], float(m), 0.0,
                            AluOp.subtract, AluOp.max,
                            accum_out=strip[:, _val_col(ch, m):
                                            _val_col(ch, m) + 1])
                    for m in CNT_DVE:
                        nc.vector.tensor_scalar(
                            trash_d[:], xc[:], float(m), 0.0,
                            AluOp.is_ge, AluOp.add,
                            accum_out=strip[:, _cnt_col(ch, m):
                                            _cnt_col(ch, m) + 1])

            # CNT_HALVES: chunk-0 half on DVE, chunk-1 half on ACT Sign
            for mh in CNT_HALVES:
                ih = N_CSPLIT.index(mh)
                for ch, xc in ((0, xp1), (1, xp2)):
                    c0 = _cnt_col(ch, mh)
                    c1 = _csplit_col(ch, ih)
                    nc.vector.tensor_scalar(
                        trash_d[:, 0:CHUNK], xc[:, 0:CHUNK], float(mh), 0.0,
                        AluOp.is_ge, AluOp.add,
                        accum_out=strip[:, c0:c0 + 1])
                    nc.scalar.activation(
                        trash_a[:, CHUNK:2 * CHUNK], xc[:, CHUNK:2 * CHUNK],
                        ACT.Sign, bias=bias_f[:, mh:mh + 1], scale=1.0,
                        accum_out=strip_a[:, c1:c1 + 1])

            nc.sync.dma_start(out[:], strip[:])
            nc.sync.dma_start(out_a[:], strip_a[:])

    nc.compile()
    nc.m = get_hw_module(nc.m)
    return nc


_NC_CACHE = None


def _get_nc():
    global _NC_CACHE
    if _NC_CACHE is None:
        _NC_CACHE = build_nc()
    return _NC_CACHE


def make_in_maps(predictions, targets):
    in_maps = []
    for k in range(N_CORES):
        b = k // CORES_PER_BATCH
        d0 = (k % CORES_PER_BATCH) * D_SH
        pr = np.ascontiguousarray(
            predictions[b, :, d0:d0 + D_SH]).reshape(C, P, F)
        tg = np.ascontiguousarray(
            targets[b, 1:, d0:d0 + D_SH]).reshape(2, P, F)
        in_maps.append({"pred": pr, "targ": tg})
    return in_maps


def decode(strips):
    """strips: list of N_CORES arrays [P, N_COLS] -> final scalar (f64)."""
    n_row_elems = float(P * F)
    n_chunk_elems = float(P * CHUNK)
    Bv = np.zeros((B, 2, NBINS))       # B_m, m = 0..32
    Ng = np.zeros((B, 2, NBINS + 1))   # N_{>=m}, m = 1..33 (33 stays 0)
    Graw = np.zeros((B, 2, NBINS))
    for k in range(N_CORES):
        b = k // CORES_PER_BATCH
        s = strips[k].astype(np.float64).sum(axis=0)   # [N_COLS]
        for ch in range(2):
            for m in range(NBINS):
                Graw[b, ch, m] += s[_val_col(ch, m)]
            for i, m in enumerate(VAL_SPLIT):
                Graw[b, ch, m] += s[_split_col(ch, i)]
            for m in CNT_DVE:
                Ng[b, ch, m - 1] += s[_cnt_col(ch, m)]
            for i, m in enumerate(N_CSPLIT):
                c0 = s[_cnt_col(ch, m)]
                c1 = s[_csplit_col(ch, i)]
                if m in CNT_ACT:
                    Ng[b, ch, m - 1] += (0.5 * (c0 + n_chunk_elems)
                                         + 0.5 * (c1 + n_chunk_elems))
                elif m in CNT_HALVES:
                    Ng[b, ch, m - 1] += c0 + 0.5 * (c1 + n_chunk_elems)
                else:
                    Ng[b, ch, m - 1] += c0 + c1
    # G_m = B_m + sum_{i>m} N_{>=i}  ->  B_m = G_m - suffix
    for b in range(B):
        for ch in range(2):
            for m in range(NBINS):
                # sum_{i>m} N_{>=i}: Ng index i-1 over i = m+1..33
                suffix_m = Ng[b, ch, m:NBINS].sum()
                Bv[b, ch, m] = Graw[b, ch, m] - suffix_m
    # P_m = B_m - B_{m+1};  C_m = N_{>=m} - N_{>=m+1}
    Pm = np.concatenate([Bv[:, :, :-1] - Bv[:, :, 1:], Bv[:, :, -1:]], axis=2)
    Cm = Ng[:, :, :NBINS - 1] - Ng[:, :, 1:NBINS]    # m = 1..32

    s_bg = Pm[:, :, 0:1]
    s_i = Pm[:, :, 1:]
    n_i = Cm
    dice = 1.0 - (2.0 * s_i + EPS) / (s_bg + s_i + n_i + EPS)
    present = (n_i > 0.5).astype(np.float64)
    per_class = (dice * present).sum(axis=(0, 2)) / np.maximum(
        present.sum(axis=(0, 2)), 1.0)
    return per_class.mean()


def kernel(predictions, targets):
    predictions = np.asarray(predictions, dtype=np.float32)
    targets = np.asarray(targets, dtype=np.int32)
    nc = _get_nc()
    in_maps = make_in_maps(predictions, targets)
    res = bass_utils.run_bass_kernel_spmd(
        nc, in_maps, core_ids=list(range(N_CORES)))
    strips = [res.results[k]["out"] + res.results[k]["out_a"]
              for k in range(N_CORES)]
    return np.float32(decode(strips))
